# revision 7
# baseline (speedup 1.0000x reference)
"""CycleVAR VQ-codebook encoder kernel for Trainium2 (8 NeuronCores).

Contract: kernel(**inputs) takes FULL inputs
  f_src      [128, 32, 16, 16] fp32
  emb_weight [4096, 32] fp32
and returns the FULL output x_var [128, 340, 32] fp32.

Observation: the reference's x_var depends only on quantization stages
pn in (1, 2, 4, 8); the pn=16 stage's outputs (stage_maps[4], f_hat,
final f_rest) are never used. So only 85 tokens/image are quantized.

Sharding: data-parallel over batch (16 images per core), codebook and
resize matrices replicated. No cross-core communication.

Per-core pipeline (all layouts per-core, B=16, C=32, H=W=16, S=256):
  f layout "spatial": [s(part, 2x128), (b,c)(free, 512)]
  stage si, pn in (1,2,4,8), P=pn^2, ntok=16P, tokens t = b*P + p:
    z-down   (PE):  z[(b,c), p] = sum_s f_rest[s, bc] * A_pn[p, s]
    zaug     (ACT): per-b copies -> zaug[0:32, b*P:(b+1)*P]; row 32 = 1.0
    scores   (PE):  s[t, v] = sum_k zaug[k, t] * eaug[k, v]  (fp32, K=33,
                    eaug rows 0..31 = emb.T, row 32 = -0.5*|e|^2)
    argmax   (DVE): grouped reduce_max (32 groups of 128) -> max8 ->
                    max_index -> winning group g*; scores roundtrip to
                    DRAM; indirect-DMA gathers row (t*32+g*) -> local
                    max_index -> exact argmax v*
    h        (DMA): indirect gather emb[v*] -> DRAM -> reload as
                    h_sp [p(part), (b,c)]
    up       (PE):  h_up[s, bc] = sum_p U_pn[s, p] * h_sp[p, bc]
    update   (DVE): f_rest -= h_up (si<3); f_partial += h_up
    x_out    (PE):  x_si[p', bc] = sum_s A_pn'[p', s] * f_partial[s, bc]
                    (si<3; si=3 emits f_partial directly)
Output DRAM x_out [340, (b,c)]; host transposes to [b, 340, c].
"""

import os

import numpy as np

import concourse.bacc as bacc
import concourse.bass as bass
import concourse.mybir as mybir
import concourse.tile as tile
from concourse.bass import IndirectOffsetOnAxis
from concourse.bass_utils import run_bass_kernel_spmd

N_CORES = 8
B_FULL = 128
B_LOC = B_FULL // N_CORES  # 16
C = 32
H = 16
S = H * H  # 256
V = 4096
PNS = (1, 2, 4, 8)
ROW_OFF = (0, 4, 20, 84)  # x_var row offsets per stage
NTOK_OUT = 340

F32 = mybir.dt.float32
U32 = mybir.dt.uint32
AX = mybir.AxisListType
ALU = mybir.AluOpType
ACTF = mybir.ActivationFunctionType

LAST_RESULTS = None  # test harness introspection


def _keys_cubic(x, a=-0.5):
    x = np.abs(x)
    return np.where(
        x <= 1,
        (a + 2) * x**3 - (a + 3) * x**2 + 1,
        np.where(x < 2, a * x**3 - 5 * a * x**2 + 8 * a * x - 4 * a, 0.0),
    )


def _resize_matrix_1d(n_in, n_out):
    # matches jax.image.resize(method='cubic') for upsampling
    scale = n_out / n_in
    U = np.zeros((n_out, n_in), np.float64)
    for i in range(n_out):
        x = (i + 0.5) / scale - 0.5
        w = _keys_cubic(x - np.arange(n_in))
        s = w.sum()
        if s != 0:
            w = w / s
        U[i] = w
    return U


def _up_matrix(pn):
    # [S, pn*pn] bicubic upsample matrix (kron of separable 1D)
    if pn == H:
        return np.eye(S, dtype=np.float32)
    U1 = _resize_matrix_1d(pn, H)
    return np.kron(U1, U1).astype(np.float32)


def _down_matrix(pn):
    # [pn*pn, S] exact area mean (r = H//pn, weight 1/r^2, exact pow2)
    r = H // pn
    A = np.zeros((pn * pn, S), np.float32)
    w = np.float32(1.0 / (r * r))
    for pi in range(pn):
        for pj in range(pn):
            for di in range(r):
                for dj in range(r):
                    A[pi * pn + pj, (pi * r + di) * H + (pj * r + dj)] = w
    return A


def _build_program():
    nc = bacc.Bacc(trn_type="TRN2", target_bir_lowering=False, debug=False)

    # DRAM I/O (per core)
    f_in = nc.dram_tensor("f_pre", [2, 128, 512], F32, kind="ExternalInput").ap()
    eaug_in = nc.dram_tensor("eaug", [33, V], F32, kind="ExternalInput").ap()
    emb_in = nc.dram_tensor("embt", [V, C], F32, kind="ExternalInput").ap()
    a_in = {
        pn: nc.dram_tensor(f"a{pn}", [2, 128, pn * pn], F32, kind="ExternalInput").ap()
        for pn in PNS
    }
    u_in = {
        pn: nc.dram_tensor(f"u{pn}", [pn * pn, 256], F32, kind="ExternalInput").ap()
        for pn in PNS
    }
    x_out = nc.dram_tensor("xout", [NTOK_OUT, 512], F32, kind="ExternalOutput").ap()

    with tile.TileContext(nc) as tc:
        from contextlib import ExitStack

        ctx = ExitStack()
        const = ctx.enter_context(tc.tile_pool(name="const", bufs=1))
        work = ctx.enter_context(tc.tile_pool(name="work", bufs=2))
        small = ctx.enter_context(tc.tile_pool(name="small", bufs=2))
        psum = ctx.enter_context(tc.tile_pool(name="psum", bufs=3, space="PSUM"))
        dram = ctx.enter_context(tc.tile_pool(name="dram", bufs=2, space="DRAM"))

        # ---- constants to SBUF ----
        eaug = const.tile([33, V], F32)
        nc.sync.dma_start(eaug, eaug_in)
        a_sb = {}
        for pn in PNS:
            P = pn * pn
            a_sb[pn] = [const.tile([128, P], F32, name=f"a{pn}_{ch}") for ch in range(2)]
            for ch in range(2):
                nc.sync.dma_start(a_sb[pn][ch], a_in[pn][ch])
        u_sb = {}
        for pn in PNS:
            P = pn * pn
            u_sb[pn] = const.tile([P, 256], F32, name=f"u{pn}")
            nc.sync.dma_start(u_sb[pn], u_in[pn])

        f_rest = [const.tile([128, 512], F32, name=f"frest{ch}") for ch in range(2)]
        f_partial = [const.tile([128, 512], F32, name=f"fpart{ch}") for ch in range(2)]
        for ch in range(2):
            nc.sync.dma_start(f_rest[ch], f_in[ch])
            nc.vector.memset(f_partial[ch], 0.0)

        zaug = const.tile([33, 1024], F32)
        nc.vector.memset(zaug[32:33, :], 1.0)

        toff32 = const.tile([128, 1], U32)  # t*32 per partition
        nc.gpsimd.iota(toff32, pattern=[[1, 1]], base=0, channel_multiplier=32)

        for si, pn in enumerate(PNS):
            P = pn * pn
            ntok = B_LOC * P
            nblk = (ntok + 127) // 128

            # ---- z-down per image: z_b[c, p] = sum_s f_rest[s, (b,c)] A[p, s] ----
            for b in range(B_LOC):
                psz = psum.tile([32, max(P, 1)], F32, tag="psz", bufs=2)
                for ch in range(2):
                    nc.tensor.matmul(
                        psz[:, :P],
                        f_rest[ch][:, 32 * b : 32 * (b + 1)],
                        a_sb[pn][ch][:, :P],
                        start=(ch == 0),
                        stop=(ch == 1),
                    )
                nc.scalar.activation(
                    zaug[0:32, b * P : (b + 1) * P], psz[:, :P], ACTF.Copy
                )

            h_d = dram.tile([ntok, C], F32, tag="hd", name=f"hd{si}")

            for blk in range(nblk):
                t0 = blk * 128
                tw = min(128, ntok - t0)

                scsb = work.tile([128, V], F32, tag="scsb")
                gmax = small.tile([128, 32], F32, tag="gmax")
                for q in range(4):
                    pq = psum.tile([128, 1024], F32, tag="psq", bufs=2)
                    for m in range(2):
                        nc.tensor.matmul(
                            pq[:tw, 512 * m : 512 * (m + 1)],
                            zaug[:, t0 : t0 + tw],
                            eaug[:, 1024 * q + 512 * m : 1024 * q + 512 * (m + 1)],
                            start=True,
                            stop=True,
                        )
                    nc.vector.reduce_max(
                        gmax[:tw, 8 * q : 8 * (q + 1)],
                        pq[:tw].rearrange("t (g s) -> t g s", g=8),
                        axis=AX.X,
                    )
                    nc.scalar.activation(
                        scsb[:tw, 1024 * q : 1024 * (q + 1)], pq[:tw], ACTF.Copy
                    )

                top8 = small.tile([128, 8], F32, tag="top8")
                gidx = small.tile([128, 8], U32, tag="gidx")
                nc.vector.max(top8[:tw], gmax[:tw])
                nc.vector.max_index(gidx[:tw], top8[:tw], gmax[:tw])

                scd = dram.tile([128 * 32, 128], F32, tag="scd")
                nc.sync.dma_start(
                    scd.rearrange("(t g) s -> t g s", g=32)[:tw],
                    scsb[:tw].rearrange("t (g s) -> t g s", g=32),
                )
                off = small.tile([128, 1], U32, tag="off")
                nc.vector.tensor_tensor(
                    off[:tw], toff32[:tw], gidx[:tw, 0:1], op=ALU.add
                )
                grp = small.tile([128, 128], F32, tag="grp")
                nc.gpsimd.indirect_dma_start(
                    grp[:tw], None, scd[:, :], IndirectOffsetOnAxis(ap=off[:tw], axis=0)
                )
                jidx = small.tile([128, 8], U32, tag="jidx")
                nc.vector.max_index(jidx[:tw], top8[:tw], grp[:tw])
                vidx = small.tile([128, 1], U32, tag="vidx")
                nc.vector.tensor_scalar_mul(vidx[:tw], gidx[:tw, 0:1], 128)
                nc.vector.tensor_tensor(
                    vidx[:tw], vidx[:tw], jidx[:tw, 0:1], op=ALU.add
                )

                htok = small.tile([128, C], F32, tag="htok")
                nc.gpsimd.indirect_dma_start(
                    htok[:tw],
                    None,
                    emb_in,
                    IndirectOffsetOnAxis(ap=vidx[:tw], axis=0),
                )
                nc.sync.dma_start(h_d[t0 : t0 + tw], htok[:tw])

            # ---- reload h as [p, (b, c)] ----
            h_sp = work.tile([max(P, 1), 512], F32, tag="hsp")
            nc.sync.dma_start(
                h_sp[:P].rearrange("p (b c) -> p b c", b=B_LOC),
                h_d.rearrange("(b p) c -> p b c", b=B_LOC),
            )

            # ---- bicubic up + residual/partial updates ----
            for ch in range(2):
                pu = psum.tile([128, 512], F32, tag="psq", bufs=2)
                nc.tensor.matmul(
                    pu,
                    u_sb[pn][:, 128 * ch : 128 * (ch + 1)],
                    h_sp[:P],
                    start=True,
                    stop=True,
                )
                if si < 3:
                    nc.vector.tensor_tensor(
                        f_rest[ch], f_rest[ch], pu, op=ALU.subtract
                    )
                nc.vector.tensor_tensor(f_partial[ch], f_partial[ch], pu, op=ALU.add)

            # ---- x output ----
            if si < 3:
                pn2 = PNS[si + 1]
                P2 = pn2 * pn2
                px = psum.tile([128, 512], F32, tag="psq", bufs=2)
                for ch in range(2):
                    nc.tensor.matmul(
                        px[:P2],
                        a_sb[pn2][ch][:, :P2],
                        f_partial[ch],
                        start=(ch == 0),
                        stop=(ch == 1),
                    )
                x_sb = small.tile([max(P2, 1), 512], F32, tag="xsb")
                nc.scalar.activation(x_sb[:P2], px[:P2], ACTF.Copy)
                nc.sync.dma_start(x_out[ROW_OFF[si] : ROW_OFF[si] + P2], x_sb[:P2])
            else:
                for ch in range(2):
                    nc.sync.dma_start(
                        x_out[84 + 128 * ch : 84 + 128 * (ch + 1)], f_partial[ch]
                    )

        ctx.close()

    nc.compile()
    return nc


_PROGRAM = None


def _get_program():
    global _PROGRAM
    if _PROGRAM is None:
        _PROGRAM = _build_program()
    return _PROGRAM


def kernel(f_src, emb_weight):
    global LAST_RESULTS
    f_src = np.asarray(f_src, dtype=np.float32)
    emb_weight = np.asarray(emb_weight, dtype=np.float32)

    e64 = emb_weight.astype(np.float64)
    eaug = np.concatenate(
        [emb_weight.T, (-0.5 * (e64 * e64).sum(1)).astype(np.float32)[None, :]], axis=0
    )  # [33, V]

    a_mats = {}
    u_mats = {}
    for pn in PNS:
        P = pn * pn
        a_mats[pn] = np.ascontiguousarray(
            _down_matrix(pn).T.reshape(2, 128, P)
        )  # [2, 128, P]
        u_mats[pn] = np.ascontiguousarray(_up_matrix(pn).T)  # [P, 256]

    in_maps = []
    for core in range(N_CORES):
        fb = f_src[core * B_LOC : (core + 1) * B_LOC]  # [16, 32, 16, 16]
        f_pre = (
            fb.reshape(B_LOC, C, S).transpose(2, 0, 1).reshape(2, 128, 512)
        )  # [s, b, c]
        m = {
            "f_pre": np.ascontiguousarray(f_pre),
            "eaug": np.ascontiguousarray(eaug),
            "embt": np.ascontiguousarray(emb_weight),
        }
        for pn in PNS:
            m[f"a{pn}"] = a_mats[pn]
            m[f"u{pn}"] = u_mats[pn]
        in_maps.append(m)

    nc = _get_program()
    trace = bool(os.environ.get("CVAR_TRACE"))
    try:
        res = run_bass_kernel_spmd(
            nc,
            in_maps,
            core_ids=list(range(N_CORES)),
            trace=trace,
        )
    except ModuleNotFoundError:
        res = run_bass_kernel_spmd(
            nc, in_maps, core_ids=list(range(N_CORES)), trace=False
        )
    LAST_RESULTS = res

    outs = []
    for core in range(N_CORES):
        xo = res.results[core]["xout"]  # [340, 512]
        outs.append(xo.reshape(NTOK_OUT, B_LOC, C).transpose(1, 0, 2))
    return np.ascontiguousarray(np.concatenate(outs, axis=0))


# revision 15
# speedup vs baseline: 1.0094x; 1.0094x over previous
"""CycleVAR VQ-codebook encoder kernel for Trainium2 (8 NeuronCores).

Contract: kernel(**inputs) takes FULL inputs
  f_src      [128, 32, 16, 16] fp32
  emb_weight [4096, 32] fp32
and returns the FULL output x_var [128, 340, 32] fp32.

Observation: the reference's x_var depends only on quantization stages
pn in (1, 2, 4, 8); the pn=16 stage's outputs (stage_maps[4], f_hat,
final f_rest) are never used. So only 85 tokens/image are quantized.

Sharding: data-parallel over batch (16 images per core), codebook and
resize matrices replicated. No cross-core communication.

Per-core pipeline (all layouts per-core, B=16, C=32, H=W=16, S=256):
  f layout "spatial": [s(part, 2x128), (b,c)(free, 512)]
  stage si, pn in (1,2,4,8), P=pn^2, ntok=16P, tokens t = b*P + p:
    z-down   (PE):  z[(b,c), p] = sum_s f_rest[s, bc] * A_pn[p, s]
    zaug     (ACT): per-b copies -> zaug[0:32, b*P:(b+1)*P]; row 32 = 1.0
    scores   (PE):  s[t, v] = sum_k zaug[k, t] * eaug[k, v]  (fp32, K=33,
                    eaug rows 0..31 = emb.T, row 32 = -0.5*|e|^2)
    argmax   (DVE): grouped reduce_max (32 groups of 128) -> max8 ->
                    max_index -> winning group g*; scores roundtrip to
                    DRAM; indirect-DMA gathers row (t*32+g*) -> local
                    max_index -> exact argmax v*
    h        (DMA): indirect gather emb[v*] -> DRAM -> reload as
                    h_sp [p(part), (b,c)]
    up       (PE):  h_up[s, bc] = sum_p U_pn[s, p] * h_sp[p, bc]
    update   (DVE): f_rest -= h_up (si<3); f_partial += h_up
    x_out    (PE):  x_si[p', bc] = sum_s A_pn'[p', s] * f_partial[s, bc]
                    (si<3; si=3 emits f_partial directly)
Output DRAM x_out [340, (b,c)]; host transposes to [b, 340, c].
"""

import os

import numpy as np

import concourse.bacc as bacc
import concourse.bass as bass
import concourse.mybir as mybir
import concourse.tile as tile
from concourse.bass import IndirectOffsetOnAxis
from concourse.bass_utils import run_bass_kernel_spmd

N_CORES = 8
B_FULL = 128
B_LOC = B_FULL // N_CORES  # 16
C = 32
H = 16
S = H * H  # 256
V = 4096
PNS = (1, 2, 4, 8)
ROW_OFF = (0, 4, 20, 84)  # x_var row offsets per stage
NTOK_OUT = 340

F32 = mybir.dt.float32
U32 = mybir.dt.uint32
AX = mybir.AxisListType
ALU = mybir.AluOpType
ACTF = mybir.ActivationFunctionType

LAST_RESULTS = None  # test harness introspection


def _keys_cubic(x, a=-0.5):
    x = np.abs(x)
    return np.where(
        x <= 1,
        (a + 2) * x**3 - (a + 3) * x**2 + 1,
        np.where(x < 2, a * x**3 - 5 * a * x**2 + 8 * a * x - 4 * a, 0.0),
    )


def _resize_matrix_1d(n_in, n_out):
    # matches jax.image.resize(method='cubic') for upsampling
    scale = n_out / n_in
    U = np.zeros((n_out, n_in), np.float64)
    for i in range(n_out):
        x = (i + 0.5) / scale - 0.5
        w = _keys_cubic(x - np.arange(n_in))
        s = w.sum()
        if s != 0:
            w = w / s
        U[i] = w
    return U


def _up_matrix(pn):
    # [S, pn*pn] bicubic upsample matrix (kron of separable 1D)
    if pn == H:
        return np.eye(S, dtype=np.float32)
    U1 = _resize_matrix_1d(pn, H)
    return np.kron(U1, U1).astype(np.float32)


def _down_matrix(pn):
    # [pn*pn, S] exact area mean (r = H//pn, weight 1/r^2, exact pow2)
    r = H // pn
    A = np.zeros((pn * pn, S), np.float32)
    w = np.float32(1.0 / (r * r))
    for pi in range(pn):
        for pj in range(pn):
            for di in range(r):
                for dj in range(r):
                    A[pi * pn + pj, (pi * r + di) * H + (pj * r + dj)] = w
    return A


def _build_program():
    nc = bacc.Bacc(trn_type="TRN2", target_bir_lowering=False, debug=False)

    # DRAM I/O (per core)
    f_in = nc.dram_tensor("f_pre", [2, 128, 512], F32, kind="ExternalInput").ap()
    eaug_in = nc.dram_tensor("eaug", [33, V], F32, kind="ExternalInput").ap()
    emb_in = nc.dram_tensor("embt", [V, C], F32, kind="ExternalInput").ap()
    a_in = {
        pn: nc.dram_tensor(f"a{pn}", [2, 128, pn * pn], F32, kind="ExternalInput").ap()
        for pn in PNS
    }
    u_in = {
        pn: nc.dram_tensor(f"u{pn}", [pn * pn, 256], F32, kind="ExternalInput").ap()
        for pn in PNS
    }
    x_out = nc.dram_tensor("xout", [NTOK_OUT, 512], F32, kind="ExternalOutput").ap()

    with tile.TileContext(nc) as tc:
        from contextlib import ExitStack

        ctx = ExitStack()
        const = ctx.enter_context(tc.tile_pool(name="const", bufs=1))
        work = ctx.enter_context(tc.tile_pool(name="work", bufs=2))
        small = ctx.enter_context(tc.tile_pool(name="small", bufs=2))
        psum = ctx.enter_context(tc.tile_pool(name="psum", bufs=3, space="PSUM"))
        dram = ctx.enter_context(tc.tile_pool(name="dram", bufs=2, space="DRAM"))

        # ---- constants to SBUF ----
        # eaug + zaug replicated at partition base 64 so score matmuls run
        # 2x-packed on PE row-groups {0,1} and {2,3} via tile_position.
        eaug_big = const.tile([97, V], F32)
        nc.sync.dma_start(eaug_big[0:33], eaug_in)
        nc.sync.dma_start(eaug_big[64:97], eaug_in)
        eaug = eaug_big[0:33]
        a_sb = {}
        for pn in PNS:
            P = pn * pn
            a_sb[pn] = [const.tile([128, P], F32, name=f"a{pn}_{ch}") for ch in range(2)]
            for ch in range(2):
                nc.sync.dma_start(a_sb[pn][ch], a_in[pn][ch])
        u_sb = {}
        for pn in PNS:
            P = pn * pn
            u_sb[pn] = const.tile([P, 256], F32, name=f"u{pn}")
            nc.sync.dma_start(u_sb[pn], u_in[pn])

        f_rest = [const.tile([128, 512], F32, name=f"frest{ch}") for ch in range(2)]
        f_partial = [const.tile([128, 512], F32, name=f"fpart{ch}") for ch in range(2)]
        for ch in range(2):
            nc.sync.dma_start(f_rest[ch], f_in[ch])
            nc.vector.memset(f_partial[ch], 0.0)

        zaug_big = const.tile([97, 1024], F32)
        zaug = zaug_big[0:33]
        nc.vector.memset(zaug_big[32:33, :], 1.0)

        toff32 = const.tile([128, 1], U32)  # t*32 per partition
        nc.gpsimd.iota(toff32, pattern=[[1, 1]], base=0, channel_multiplier=32)

        for si, pn in enumerate(PNS):
            P = pn * pn
            ntok = B_LOC * P
            nblk = (ntok + 127) // 128

            # ---- z-down per image: z_b[c, p] = sum_s f_rest[s, (b,c)] A[p, s] ----
            for b in range(B_LOC):
                psz = psum.tile([32, max(P, 1)], F32, tag="psz", bufs=2)
                for ch in range(2):
                    nc.tensor.matmul(
                        psz[:, :P],
                        f_rest[ch][:, 32 * b : 32 * (b + 1)],
                        a_sb[pn][ch][:, :P],
                        start=(ch == 0),
                        stop=(ch == 1),
                    )
                nc.scalar.activation(
                    zaug[0:32, b * P : (b + 1) * P], psz[:, :P], ACTF.Copy
                )
            # replicate tokens (and the ones row) to partition base 64
            nc.sync.dma_start(zaug_big[64:97, :ntok], zaug_big[0:33, :ntok])

            h_sp = work.tile([max(P, 1), 512], F32, tag="hsp", name=f"hsp{si}")

            for blk in range(nblk):
                t0 = blk * 128
                tw = min(128, ntok - t0)

                scsb = work.tile([128, V], F32, tag="scsb")
                gmax = small.tile([128, 32], F32, tag="gmax")
                for q in range(4):
                    # 2x-packed: codes [512q, 512q+512) on row-groups 0-1,
                    # codes [2048+512q, ...) on row-groups 2-3 concurrently
                    pq = psum.tile([128, 1024], F32, tag="psq", bufs=3)
                    nc.tensor.matmul(
                        pq[:tw, 0:512],
                        zaug_big[0:33, t0 : t0 + tw],
                        eaug_big[0:33, 512 * q : 512 * (q + 1)],
                        start=True,
                        stop=True,
                        tile_position=(0, 0),
                    )
                    nc.tensor.matmul(
                        pq[:tw, 512:1024],
                        zaug_big[64:97, t0 : t0 + tw],
                        eaug_big[64:97, 2048 + 512 * q : 2048 + 512 * (q + 1)],
                        start=True,
                        stop=True,
                        tile_position=(64, 0),
                    )
                    nc.vector.reduce_max(
                        gmax[:tw, 4 * q : 4 * (q + 1)],
                        pq[:tw, 0:512].rearrange("t (g s) -> t g s", g=4),
                        axis=AX.X,
                    )
                    nc.vector.reduce_max(
                        gmax[:tw, 16 + 4 * q : 16 + 4 * (q + 1)],
                        pq[:tw, 512:1024].rearrange("t (g s) -> t g s", g=4),
                        axis=AX.X,
                    )
                    nc.scalar.activation(
                        scsb[:tw, 512 * q : 512 * (q + 1)], pq[:tw, 0:512], ACTF.Copy
                    )
                    nc.scalar.activation(
                        scsb[:tw, 2048 + 512 * q : 2048 + 512 * (q + 1)],
                        pq[:tw, 512:1024],
                        ACTF.Copy,
                    )

                top8 = small.tile([128, 8], F32, tag="top8")
                gidx = small.tile([128, 8], U32, tag="gidx")
                nc.vector.max(top8[:tw], gmax[:tw])
                nc.vector.max_index(gidx[:tw], top8[:tw], gmax[:tw])

                scd = dram.tile([128 * 32, 128], F32, tag="scd")
                nc.sync.dma_start(
                    scd.rearrange("(t g) s -> t g s", g=32)[:tw],
                    scsb[:tw].rearrange("t (g s) -> t g s", g=32),
                )
                off = small.tile([128, 1], U32, tag="off")
                nc.vector.tensor_tensor(
                    off[:tw], toff32[:tw], gidx[:tw, 0:1], op=ALU.add
                )
                grp = small.tile([128, 128], F32, tag="grp")
                nc.gpsimd.indirect_dma_start(
                    grp[:tw], None, scd[:, :], IndirectOffsetOnAxis(ap=off[:tw], axis=0)
                )
                jidx = small.tile([128, 8], U32, tag="jidx")
                nc.vector.max_index(jidx[:tw], top8[:tw], grp[:tw])
                vidx = small.tile([128, 1], U32, tag="vidx")
                nc.vector.tensor_scalar_mul(vidx[:tw], gidx[:tw, 0:1], 128)
                nc.vector.tensor_tensor(
                    vidx[:tw], vidx[:tw], jidx[:tw, 0:1], op=ALU.add
                )

                htok = small.tile([128, C], F32, tag="htok")
                nc.gpsimd.indirect_dma_start(
                    htok[:tw],
                    None,
                    emb_in,
                    IndirectOffsetOnAxis(ap=vidx[:tw], axis=0),
                )
                # scatter into h_sp [p, (b,c)]: per-b partition-shift copies
                for b in range(t0 // P, (t0 + tw) // P):
                    r0 = b * P - t0
                    nc.sync.dma_start(
                        h_sp[0:P, 32 * b : 32 * (b + 1)], htok[r0 : r0 + P]
                    )

            # ---- bicubic up + residual/partial updates ----
            for ch in range(2):
                pu = psum.tile([128, 512], F32, tag="psq", bufs=3)
                nc.tensor.matmul(
                    pu,
                    u_sb[pn][:, 128 * ch : 128 * (ch + 1)],
                    h_sp[:P],
                    start=True,
                    stop=True,
                )
                if si < 3:
                    nc.vector.tensor_tensor(
                        f_rest[ch], f_rest[ch], pu, op=ALU.subtract
                    )
                nc.vector.tensor_tensor(f_partial[ch], f_partial[ch], pu, op=ALU.add)

            # ---- x output ----
            if si < 3:
                pn2 = PNS[si + 1]
                P2 = pn2 * pn2
                px = psum.tile([128, 512], F32, tag="psq", bufs=3)
                for ch in range(2):
                    nc.tensor.matmul(
                        px[:P2],
                        a_sb[pn2][ch][:, :P2],
                        f_partial[ch],
                        start=(ch == 0),
                        stop=(ch == 1),
                    )
                x_sb = small.tile([max(P2, 1), 512], F32, tag="xsb")
                nc.scalar.activation(x_sb[:P2], px[:P2], ACTF.Copy)
                nc.sync.dma_start(x_out[ROW_OFF[si] : ROW_OFF[si] + P2], x_sb[:P2])
            else:
                for ch in range(2):
                    nc.sync.dma_start(
                        x_out[84 + 128 * ch : 84 + 128 * (ch + 1)], f_partial[ch]
                    )

        ctx.close()

    nc.compile()
    return nc


_PROGRAM = None


def _get_program():
    global _PROGRAM
    if _PROGRAM is None:
        _PROGRAM = _build_program()
    return _PROGRAM


def kernel(f_src, emb_weight):
    global LAST_RESULTS
    f_src = np.asarray(f_src, dtype=np.float32)
    emb_weight = np.asarray(emb_weight, dtype=np.float32)

    e64 = emb_weight.astype(np.float64)
    eaug = np.concatenate(
        [emb_weight.T, (-0.5 * (e64 * e64).sum(1)).astype(np.float32)[None, :]], axis=0
    )  # [33, V]

    a_mats = {}
    u_mats = {}
    for pn in PNS:
        P = pn * pn
        a_mats[pn] = np.ascontiguousarray(
            _down_matrix(pn).T.reshape(2, 128, P)
        )  # [2, 128, P]
        u_mats[pn] = np.ascontiguousarray(_up_matrix(pn).T)  # [P, 256]

    in_maps = []
    for core in range(N_CORES):
        fb = f_src[core * B_LOC : (core + 1) * B_LOC]  # [16, 32, 16, 16]
        f_pre = (
            fb.reshape(B_LOC, C, S).transpose(2, 0, 1).reshape(2, 128, 512)
        )  # [s, b, c]
        m = {
            "f_pre": np.ascontiguousarray(f_pre),
            "eaug": np.ascontiguousarray(eaug),
            "embt": np.ascontiguousarray(emb_weight),
        }
        for pn in PNS:
            m[f"a{pn}"] = a_mats[pn]
            m[f"u{pn}"] = u_mats[pn]
        in_maps.append(m)

    nc = _get_program()
    trace = bool(os.environ.get("CVAR_TRACE"))
    try:
        res = run_bass_kernel_spmd(
            nc,
            in_maps,
            core_ids=list(range(N_CORES)),
            trace=trace,
        )
    except ModuleNotFoundError:
        res = run_bass_kernel_spmd(
            nc, in_maps, core_ids=list(range(N_CORES)), trace=False
        )
    LAST_RESULTS = res

    outs = []
    for core in range(N_CORES):
        xo = res.results[core]["xout"]  # [340, 512]
        outs.append(xo.reshape(NTOK_OUT, B_LOC, C).transpose(1, 0, 2))
    return np.ascontiguousarray(np.concatenate(outs, axis=0))


# revision 17
# speedup vs baseline: 1.1137x; 1.1033x over previous
"""CycleVAR VQ-codebook encoder kernel for Trainium2 (8 NeuronCores).

Contract: kernel(**inputs) takes FULL inputs
  f_src      [128, 32, 16, 16] fp32
  emb_weight [4096, 32] fp32
and returns the FULL output x_var [128, 340, 32] fp32.

Observation: the reference's x_var depends only on quantization stages
pn in (1, 2, 4, 8); the pn=16 stage's outputs (stage_maps[4], f_hat,
final f_rest) are never used. So only 85 tokens/image are quantized.

Sharding: data-parallel over batch (16 images per core), codebook and
resize matrices replicated. No cross-core communication.

Per-core pipeline (all layouts per-core, B=16, C=32, H=W=16, S=256):
  f layout "spatial": [s(part, 2x128), (b,c)(free, 512)]
  stage si, pn in (1,2,4,8), P=pn^2, ntok=16P, tokens t = b*P + p:
    z-down   (PE):  z[(b,c), p] = sum_s f_rest[s, bc] * A_pn[p, s]
    zaug     (ACT): per-b copies -> zaug[0:32, b*P:(b+1)*P]; row 32 = 1.0
    scores   (PE):  s[t, v] = sum_k zaug[k, t] * eaug[k, v]  (fp32, K=33,
                    eaug rows 0..31 = emb.T, row 32 = -0.5*|e|^2)
    argmax   (DVE): grouped reduce_max (32 groups of 128) -> max8 ->
                    max_index -> winning group g*; scores roundtrip to
                    DRAM; indirect-DMA gathers row (t*32+g*) -> local
                    max_index -> exact argmax v*
    h        (DMA): indirect gather emb[v*] -> DRAM -> reload as
                    h_sp [p(part), (b,c)]
    up       (PE):  h_up[s, bc] = sum_p U_pn[s, p] * h_sp[p, bc]
    update   (DVE): f_rest -= h_up (si<3); f_partial += h_up
    x_out    (PE):  x_si[p', bc] = sum_s A_pn'[p', s] * f_partial[s, bc]
                    (si<3; si=3 emits f_partial directly)
Output DRAM x_out [340, (b,c)]; host transposes to [b, 340, c].
"""

import os

import numpy as np

import concourse.bacc as bacc
import concourse.bass as bass
import concourse.mybir as mybir
import concourse.tile as tile
from concourse.bass import IndirectOffsetOnAxis
from concourse.bass_utils import run_bass_kernel_spmd

N_CORES = 8
B_FULL = 128
B_LOC = B_FULL // N_CORES  # 16
C = 32
H = 16
S = H * H  # 256
V = 4096
PNS = (1, 2, 4, 8)
ROW_OFF = (0, 4, 20, 84)  # x_var row offsets per stage
NTOK_OUT = 340

F32 = mybir.dt.float32
U32 = mybir.dt.uint32
AX = mybir.AxisListType
ALU = mybir.AluOpType
ACTF = mybir.ActivationFunctionType

LAST_RESULTS = None  # test harness introspection


def _keys_cubic(x, a=-0.5):
    x = np.abs(x)
    return np.where(
        x <= 1,
        (a + 2) * x**3 - (a + 3) * x**2 + 1,
        np.where(x < 2, a * x**3 - 5 * a * x**2 + 8 * a * x - 4 * a, 0.0),
    )


def _resize_matrix_1d(n_in, n_out):
    # matches jax.image.resize(method='cubic') for upsampling
    scale = n_out / n_in
    U = np.zeros((n_out, n_in), np.float64)
    for i in range(n_out):
        x = (i + 0.5) / scale - 0.5
        w = _keys_cubic(x - np.arange(n_in))
        s = w.sum()
        if s != 0:
            w = w / s
        U[i] = w
    return U


def _up_matrix(pn):
    # [S, pn*pn] bicubic upsample matrix (kron of separable 1D)
    if pn == H:
        return np.eye(S, dtype=np.float32)
    U1 = _resize_matrix_1d(pn, H)
    return np.kron(U1, U1).astype(np.float32)


def _down_matrix(pn):
    # [pn*pn, S] exact area mean (r = H//pn, weight 1/r^2, exact pow2)
    r = H // pn
    A = np.zeros((pn * pn, S), np.float32)
    w = np.float32(1.0 / (r * r))
    for pi in range(pn):
        for pj in range(pn):
            for di in range(r):
                for dj in range(r):
                    A[pi * pn + pj, (pi * r + di) * H + (pj * r + dj)] = w
    return A


def _build_program():
    nc = bacc.Bacc(trn_type="TRN2", target_bir_lowering=False, debug=False)

    # DRAM I/O (per core)
    f_in = nc.dram_tensor("f_pre", [2, 128, 512], F32, kind="ExternalInput").ap()
    eaug_in = nc.dram_tensor("eaug", [33, V], F32, kind="ExternalInput").ap()
    emb_in = nc.dram_tensor("embt", [V, C], F32, kind="ExternalInput").ap()
    a_in = {
        pn: nc.dram_tensor(f"a{pn}", [2, 128, pn * pn], F32, kind="ExternalInput").ap()
        for pn in PNS
    }
    u_in = {
        pn: nc.dram_tensor(f"u{pn}", [pn * pn, 256], F32, kind="ExternalInput").ap()
        for pn in PNS
    }
    x_out = nc.dram_tensor("xout", [NTOK_OUT, 512], F32, kind="ExternalOutput").ap()

    with tile.TileContext(nc) as tc:
        from contextlib import ExitStack

        ctx = ExitStack()
        const = ctx.enter_context(tc.tile_pool(name="const", bufs=1))
        work = ctx.enter_context(tc.tile_pool(name="work", bufs=2))
        small = ctx.enter_context(tc.tile_pool(name="small", bufs=2))
        psum = ctx.enter_context(tc.tile_pool(name="psum", bufs=3, space="PSUM"))
        dram = ctx.enter_context(tc.tile_pool(name="dram", bufs=2, space="DRAM"))

        # ---- constants to SBUF ----
        # eaug + zaug replicated at partition base 64 so score matmuls run
        # 2x-packed on PE row-groups {0,1} and {2,3} via tile_position.
        eaug_big = const.tile([97, V], F32)
        nc.sync.dma_start(eaug_big[0:33], eaug_in)
        nc.sync.dma_start(eaug_big[64:97], eaug_in)
        eaug = eaug_big[0:33]
        a_sb = {}
        for pn in PNS:
            P = pn * pn
            a_sb[pn] = [const.tile([128, P], F32, name=f"a{pn}_{ch}") for ch in range(2)]
            for ch in range(2):
                nc.sync.dma_start(a_sb[pn][ch], a_in[pn][ch])
        u_sb = {}
        for pn in PNS:
            P = pn * pn
            u_sb[pn] = const.tile([P, 256], F32, name=f"u{pn}")
            nc.sync.dma_start(u_sb[pn], u_in[pn])

        f_rest = [const.tile([128, 512], F32, name=f"frest{ch}") for ch in range(2)]
        f_partial = [const.tile([128, 512], F32, name=f"fpart{ch}") for ch in range(2)]
        for ch in range(2):
            nc.sync.dma_start(f_rest[ch], f_in[ch])
            nc.vector.memset(f_partial[ch], 0.0)

        zaug_big = const.tile([97, 1024], F32)
        zaug = zaug_big[0:33]
        nc.vector.memset(zaug_big[32:33, :], 1.0)

        toff32 = const.tile([128, 1], U32)  # t*32 per partition
        nc.gpsimd.iota(toff32, pattern=[[1, 1]], base=0, channel_multiplier=32)

        for si, pn in enumerate(PNS):
            P = pn * pn
            ntok = B_LOC * P
            nblk = (ntok + 127) // 128

            # ---- z-down per image: z_b[c, p] = sum_s f_rest[s, (b,c)] A[p, s] ----
            for b in range(B_LOC):
                psz = psum.tile([32, max(P, 1)], F32, tag="psz", bufs=2)
                for ch in range(2):
                    nc.tensor.matmul(
                        psz[:, :P],
                        f_rest[ch][:, 32 * b : 32 * (b + 1)],
                        a_sb[pn][ch][:, :P],
                        start=(ch == 0),
                        stop=(ch == 1),
                    )
                nc.scalar.activation(
                    zaug[0:32, b * P : (b + 1) * P], psz[:, :P], ACTF.Copy
                )
            # replicate tokens (and the ones row) to partition base 64
            nc.sync.dma_start(zaug_big[64:97, :ntok], zaug_big[0:33, :ntok])

            h_sp = work.tile([max(P, 1), 512], F32, tag="hsp", name=f"hsp{si}")

            for blk in range(nblk):
                t0 = blk * 128
                tw = min(128, ntok - t0)

                scsb = work.tile([128, V], F32, tag="scsb", bufs=3)
                gmax = small.tile([128, 32], F32, tag="gmax")
                scd = dram.tile([128 * 32, 128], F32, tag="scd")
                for q in range(4):
                    # 2x-packed: codes [512q, 512q+512) on row-groups 0-1,
                    # codes [2048+512q, ...) on row-groups 2-3 concurrently
                    pq = psum.tile([128, 1024], F32, tag="psq", bufs=3)
                    nc.tensor.matmul(
                        pq[:tw, 0:512],
                        zaug_big[0:33, t0 : t0 + tw],
                        eaug_big[0:33, 512 * q : 512 * (q + 1)],
                        start=True,
                        stop=True,
                        tile_position=(0, 0),
                    )
                    nc.tensor.matmul(
                        pq[:tw, 512:1024],
                        zaug_big[64:97, t0 : t0 + tw],
                        eaug_big[64:97, 2048 + 512 * q : 2048 + 512 * (q + 1)],
                        start=True,
                        stop=True,
                        tile_position=(64, 0),
                    )
                    nc.vector.reduce_max(
                        gmax[:tw, 4 * q : 4 * (q + 1)],
                        pq[:tw, 0:512].rearrange("t (g s) -> t g s", g=4),
                        axis=AX.X,
                    )
                    nc.vector.reduce_max(
                        gmax[:tw, 16 + 4 * q : 16 + 4 * (q + 1)],
                        pq[:tw, 512:1024].rearrange("t (g s) -> t g s", g=4),
                        axis=AX.X,
                    )
                    nc.scalar.activation(
                        scsb[:tw, 512 * q : 512 * (q + 1)], pq[:tw, 0:512], ACTF.Copy
                    )
                    nc.scalar.activation(
                        scsb[:tw, 2048 + 512 * q : 2048 + 512 * (q + 1)],
                        pq[:tw, 512:1024],
                        ACTF.Copy,
                    )
                    nc.sync.dma_start(
                        scd.rearrange("(t g) s -> t g s", g=32)[:tw, 4 * q : 4 * q + 4],
                        scsb[:tw, 512 * q : 512 * (q + 1)].rearrange(
                            "t (g s) -> t g s", g=4
                        ),
                    )
                    nc.sync.dma_start(
                        scd.rearrange("(t g) s -> t g s", g=32)[
                            :tw, 16 + 4 * q : 16 + 4 * q + 4
                        ],
                        scsb[:tw, 2048 + 512 * q : 2048 + 512 * (q + 1)].rearrange(
                            "t (g s) -> t g s", g=4
                        ),
                    )

                top8 = small.tile([128, 8], F32, tag="top8")
                gidx = small.tile([128, 8], U32, tag="gidx")
                nc.vector.max(top8[:tw], gmax[:tw])
                nc.vector.max_index(gidx[:tw], top8[:tw], gmax[:tw])

                off = small.tile([128, 1], U32, tag="off")
                nc.vector.tensor_tensor(
                    off[:tw], toff32[:tw], gidx[:tw, 0:1], op=ALU.add
                )
                grp = small.tile([128, 128], F32, tag="grp")
                nc.gpsimd.indirect_dma_start(
                    grp[:tw], None, scd[:, :], IndirectOffsetOnAxis(ap=off[:tw], axis=0)
                )
                jidx = small.tile([128, 8], U32, tag="jidx")
                nc.vector.max_index(jidx[:tw], top8[:tw], grp[:tw])
                vidx = small.tile([128, 1], U32, tag="vidx")
                nc.vector.tensor_scalar_mul(vidx[:tw], gidx[:tw, 0:1], 128)
                nc.vector.tensor_tensor(
                    vidx[:tw], vidx[:tw], jidx[:tw, 0:1], op=ALU.add
                )

                htok = small.tile([128, C], F32, tag="htok")
                nc.gpsimd.indirect_dma_start(
                    htok[:tw],
                    None,
                    emb_in,
                    IndirectOffsetOnAxis(ap=vidx[:tw], axis=0),
                )
                # scatter into h_sp [p, (b,c)]: per-b partition-shift copies
                for b in range(t0 // P, (t0 + tw) // P):
                    r0 = b * P - t0
                    nc.sync.dma_start(
                        h_sp[0:P, 32 * b : 32 * (b + 1)], htok[r0 : r0 + P]
                    )

            # ---- bicubic up + residual/partial updates ----
            for ch in range(2):
                pu = psum.tile([128, 512], F32, tag="psq", bufs=3)
                nc.tensor.matmul(
                    pu,
                    u_sb[pn][:, 128 * ch : 128 * (ch + 1)],
                    h_sp[:P],
                    start=True,
                    stop=True,
                )
                if si < 3:
                    nc.vector.tensor_tensor(
                        f_rest[ch], f_rest[ch], pu, op=ALU.subtract
                    )
                nc.vector.tensor_tensor(f_partial[ch], f_partial[ch], pu, op=ALU.add)

            # ---- x output ----
            if si < 3:
                pn2 = PNS[si + 1]
                P2 = pn2 * pn2
                px = psum.tile([128, 512], F32, tag="psq", bufs=3)
                for ch in range(2):
                    nc.tensor.matmul(
                        px[:P2],
                        a_sb[pn2][ch][:, :P2],
                        f_partial[ch],
                        start=(ch == 0),
                        stop=(ch == 1),
                    )
                x_sb = small.tile([max(P2, 1), 512], F32, tag="xsb")
                nc.scalar.activation(x_sb[:P2], px[:P2], ACTF.Copy)
                nc.sync.dma_start(x_out[ROW_OFF[si] : ROW_OFF[si] + P2], x_sb[:P2])
            else:
                for ch in range(2):
                    nc.sync.dma_start(
                        x_out[84 + 128 * ch : 84 + 128 * (ch + 1)], f_partial[ch]
                    )

        ctx.close()

    nc.compile()
    return nc


_PROGRAM = None


def _get_program():
    global _PROGRAM
    if _PROGRAM is None:
        _PROGRAM = _build_program()
    return _PROGRAM


def kernel(f_src, emb_weight):
    global LAST_RESULTS
    f_src = np.asarray(f_src, dtype=np.float32)
    emb_weight = np.asarray(emb_weight, dtype=np.float32)

    e64 = emb_weight.astype(np.float64)
    eaug = np.concatenate(
        [emb_weight.T, (-0.5 * (e64 * e64).sum(1)).astype(np.float32)[None, :]], axis=0
    )  # [33, V]

    a_mats = {}
    u_mats = {}
    for pn in PNS:
        P = pn * pn
        a_mats[pn] = np.ascontiguousarray(
            _down_matrix(pn).T.reshape(2, 128, P)
        )  # [2, 128, P]
        u_mats[pn] = np.ascontiguousarray(_up_matrix(pn).T)  # [P, 256]

    in_maps = []
    for core in range(N_CORES):
        fb = f_src[core * B_LOC : (core + 1) * B_LOC]  # [16, 32, 16, 16]
        f_pre = (
            fb.reshape(B_LOC, C, S).transpose(2, 0, 1).reshape(2, 128, 512)
        )  # [s, b, c]
        m = {
            "f_pre": np.ascontiguousarray(f_pre),
            "eaug": np.ascontiguousarray(eaug),
            "embt": np.ascontiguousarray(emb_weight),
        }
        for pn in PNS:
            m[f"a{pn}"] = a_mats[pn]
            m[f"u{pn}"] = u_mats[pn]
        in_maps.append(m)

    nc = _get_program()
    trace = bool(os.environ.get("CVAR_TRACE"))
    try:
        res = run_bass_kernel_spmd(
            nc,
            in_maps,
            core_ids=list(range(N_CORES)),
            trace=trace,
        )
    except ModuleNotFoundError:
        res = run_bass_kernel_spmd(
            nc, in_maps, core_ids=list(range(N_CORES)), trace=False
        )
    LAST_RESULTS = res

    outs = []
    for core in range(N_CORES):
        xo = res.results[core]["xout"]  # [340, 512]
        outs.append(xo.reshape(NTOK_OUT, B_LOC, C).transpose(1, 0, 2))
    return np.ascontiguousarray(np.concatenate(outs, axis=0))


# revision 19
# speedup vs baseline: 1.1707x; 1.0512x over previous
"""CycleVAR VQ-codebook encoder kernel for Trainium2 (8 NeuronCores).

Contract: kernel(**inputs) takes FULL inputs
  f_src      [128, 32, 16, 16] fp32
  emb_weight [4096, 32] fp32
and returns the FULL output x_var [128, 340, 32] fp32.

Observation: the reference's x_var depends only on quantization stages
pn in (1, 2, 4, 8); the pn=16 stage's outputs (stage_maps[4], f_hat,
final f_rest) are never used. So only 85 tokens/image are quantized.

Sharding: data-parallel over batch (16 images per core), codebook and
resize matrices replicated. No cross-core communication.

Per-core pipeline (all layouts per-core, B=16, C=32, H=W=16, S=256):
  f layout "spatial": [s(part, 2x128), (b,c)(free, 512)]
  stage si, pn in (1,2,4,8), P=pn^2, ntok=16P, tokens t = b*P + p:
    z-down   (PE):  z[(b,c), p] = sum_s f_rest[s, bc] * A_pn[p, s]
    zaug     (ACT): per-b copies -> zaug[0:32, b*P:(b+1)*P]; row 32 = 1.0
    scores   (PE):  s[t, v] = sum_k zaug[k, t] * eaug[k, v]  (fp32, K=33,
                    eaug rows 0..31 = emb.T, row 32 = -0.5*|e|^2)
    argmax   (DVE): grouped reduce_max (32 groups of 128) -> max8 ->
                    max_index -> winning group g*; scores roundtrip to
                    DRAM; indirect-DMA gathers row (t*32+g*) -> local
                    max_index -> exact argmax v*
    h        (DMA): indirect gather emb[v*] -> DRAM -> reload as
                    h_sp [p(part), (b,c)]
    up       (PE):  h_up[s, bc] = sum_p U_pn[s, p] * h_sp[p, bc]
    update   (DVE): f_rest -= h_up (si<3); f_partial += h_up
    x_out    (PE):  x_si[p', bc] = sum_s A_pn'[p', s] * f_partial[s, bc]
                    (si<3; si=3 emits f_partial directly)
Output DRAM x_out [340, (b,c)]; host transposes to [b, 340, c].
"""

import os

import numpy as np

import concourse.bacc as bacc
import concourse.bass as bass
import concourse.mybir as mybir
import concourse.tile as tile
from concourse.bass import IndirectOffsetOnAxis
from concourse.bass_utils import run_bass_kernel_spmd

N_CORES = 8
B_FULL = 128
B_LOC = B_FULL // N_CORES  # 16
C = 32
H = 16
S = H * H  # 256
V = 4096
PNS = (1, 2, 4, 8)
ROW_OFF = (0, 4, 20, 84)  # x_var row offsets per stage
NTOK_OUT = 340

F32 = mybir.dt.float32
U32 = mybir.dt.uint32
AX = mybir.AxisListType
ALU = mybir.AluOpType
ACTF = mybir.ActivationFunctionType

LAST_RESULTS = None  # test harness introspection


def _keys_cubic(x, a=-0.5):
    x = np.abs(x)
    return np.where(
        x <= 1,
        (a + 2) * x**3 - (a + 3) * x**2 + 1,
        np.where(x < 2, a * x**3 - 5 * a * x**2 + 8 * a * x - 4 * a, 0.0),
    )


def _resize_matrix_1d(n_in, n_out):
    # matches jax.image.resize(method='cubic') for upsampling
    scale = n_out / n_in
    U = np.zeros((n_out, n_in), np.float64)
    for i in range(n_out):
        x = (i + 0.5) / scale - 0.5
        w = _keys_cubic(x - np.arange(n_in))
        s = w.sum()
        if s != 0:
            w = w / s
        U[i] = w
    return U


def _up_matrix(pn):
    # [S, pn*pn] bicubic upsample matrix (kron of separable 1D)
    if pn == H:
        return np.eye(S, dtype=np.float32)
    U1 = _resize_matrix_1d(pn, H)
    return np.kron(U1, U1).astype(np.float32)


def _down_matrix(pn):
    # [pn*pn, S] exact area mean (r = H//pn, weight 1/r^2, exact pow2)
    r = H // pn
    A = np.zeros((pn * pn, S), np.float32)
    w = np.float32(1.0 / (r * r))
    for pi in range(pn):
        for pj in range(pn):
            for di in range(r):
                for dj in range(r):
                    A[pi * pn + pj, (pi * r + di) * H + (pj * r + dj)] = w
    return A


def _build_program():
    nc = bacc.Bacc(trn_type="TRN2", target_bir_lowering=False, debug=False)

    # DRAM I/O (per core)
    f_in = nc.dram_tensor("f_pre", [2, 128, 512], F32, kind="ExternalInput").ap()
    eaug_in = nc.dram_tensor("eaug", [33, V], F32, kind="ExternalInput").ap()
    emb_in = nc.dram_tensor("embt", [V, C], F32, kind="ExternalInput").ap()
    a_in = {
        pn: nc.dram_tensor(f"a{pn}", [2, 128, pn * pn], F32, kind="ExternalInput").ap()
        for pn in PNS
    }
    u_in = {
        pn: nc.dram_tensor(f"u{pn}", [pn * pn, 256], F32, kind="ExternalInput").ap()
        for pn in PNS
    }
    x_out = nc.dram_tensor("xout", [NTOK_OUT, 512], F32, kind="ExternalOutput").ap()

    with tile.TileContext(nc) as tc:
        from contextlib import ExitStack

        ctx = ExitStack()
        const = ctx.enter_context(tc.tile_pool(name="const", bufs=1))
        work = ctx.enter_context(tc.tile_pool(name="work", bufs=2))
        small = ctx.enter_context(tc.tile_pool(name="small", bufs=2))
        psum = ctx.enter_context(tc.tile_pool(name="psum", bufs=3, space="PSUM"))
        dram = ctx.enter_context(tc.tile_pool(name="dram", bufs=2, space="DRAM"))

        # ---- constants to SBUF ----
        # eaug + zaug replicated at partition base 64 so score matmuls run
        # 2x-packed on PE row-groups {0,1} and {2,3} via tile_position.
        eaug_big = const.tile([97, V], F32)
        nc.sync.dma_start(eaug_big[0:33], eaug_in)
        nc.sync.dma_start(eaug_big[64:97], eaug_in)
        eaug = eaug_big[0:33]
        a_sb = {}
        for pn in PNS:
            P = pn * pn
            a_sb[pn] = [const.tile([128, P], F32, name=f"a{pn}_{ch}") for ch in range(2)]
            for ch in range(2):
                nc.sync.dma_start(a_sb[pn][ch], a_in[pn][ch])
        u_sb = {}
        for pn in PNS:
            P = pn * pn
            u_sb[pn] = const.tile([P, 256], F32, name=f"u{pn}")
            nc.sync.dma_start(u_sb[pn], u_in[pn])

        f_rest = [const.tile([128, 512], F32, name=f"frest{ch}") for ch in range(2)]
        f_partial = [const.tile([128, 512], F32, name=f"fpart{ch}") for ch in range(2)]
        for ch in range(2):
            nc.sync.dma_start(f_rest[ch], f_in[ch])
            nc.vector.memset(f_partial[ch], 0.0)

        zaug_big = const.tile([97, 1024], F32)
        zaug = zaug_big[0:33]
        nc.vector.memset(zaug_big[32:33, :], 1.0)

        toff32 = const.tile([128, 1], U32)  # t*32 per partition
        nc.gpsimd.iota(toff32, pattern=[[1, 1]], base=0, channel_multiplier=32)

        for si, pn in enumerate(PNS):
            P = pn * pn
            ntok = B_LOC * P
            nblk = (ntok + 127) // 128

            # ---- z-down per image: z_b[c, p] = sum_s f_rest[s, (b,c)] A[p, s] ----
            for b in range(B_LOC):
                psz = psum.tile([32, max(P, 1)], F32, tag="psz", bufs=2)
                for ch in range(2):
                    nc.tensor.matmul(
                        psz[:, :P],
                        f_rest[ch][:, 32 * b : 32 * (b + 1)],
                        a_sb[pn][ch][:, :P],
                        start=(ch == 0),
                        stop=(ch == 1),
                    )
                nc.scalar.activation(
                    zaug[0:32, b * P : (b + 1) * P], psz[:, :P], ACTF.Copy
                )
            # replicate tokens (and the ones row) to partition base 64
            nc.sync.dma_start(zaug_big[64:97, :ntok], zaug_big[0:33, :ntok])

            h_sp = work.tile([max(P, 1), 512], F32, tag="hsp", name=f"hsp{si}")

            for blk in range(nblk):
                t0 = blk * 128
                tw = min(128, ntok - t0)

                scsb = work.tile([128, V], F32, tag="scsb", bufs=3)
                gmax = small.tile([128, 32], F32, tag="gmax")
                scd = dram.tile([128 * 32, 128], F32, tag="scd")
                for q in range(4):
                    # 2x-packed: codes [512q, 512q+512) on row-groups 0-1,
                    # codes [2048+512q, ...) on row-groups 2-3 concurrently
                    pq = psum.tile([128, 1024], F32, tag="psq", bufs=3)
                    nc.tensor.matmul(
                        pq[:tw, 0:512],
                        zaug_big[0:33, t0 : t0 + tw],
                        eaug_big[0:33, 512 * q : 512 * (q + 1)],
                        start=True,
                        stop=True,
                        tile_position=(0, 0),
                    )
                    nc.tensor.matmul(
                        pq[:tw, 512:1024],
                        zaug_big[64:97, t0 : t0 + tw],
                        eaug_big[64:97, 2048 + 512 * q : 2048 + 512 * (q + 1)],
                        start=True,
                        stop=True,
                        tile_position=(64, 0),
                    )
                    nc.vector.reduce_max(
                        gmax[:tw, 4 * q : 4 * (q + 1)],
                        pq[:tw, 0:512].rearrange("t (g s) -> t g s", g=4),
                        axis=AX.X,
                    )
                    nc.vector.reduce_max(
                        gmax[:tw, 16 + 4 * q : 16 + 4 * (q + 1)],
                        pq[:tw, 512:1024].rearrange("t (g s) -> t g s", g=4),
                        axis=AX.X,
                    )
                    nc.scalar.activation(
                        scsb[:tw, 512 * q : 512 * (q + 1)], pq[:tw, 0:512], ACTF.Copy
                    )
                    nc.scalar.activation(
                        scsb[:tw, 2048 + 512 * q : 2048 + 512 * (q + 1)],
                        pq[:tw, 512:1024],
                        ACTF.Copy,
                    )
                    nc.sync.dma_start(
                        scd.rearrange("(t g) s -> t g s", g=32)[:tw, 4 * q : 4 * q + 4],
                        scsb[:tw, 512 * q : 512 * (q + 1)].rearrange(
                            "t (g s) -> t g s", g=4
                        ),
                    )
                    nc.sync.dma_start(
                        scd.rearrange("(t g) s -> t g s", g=32)[
                            :tw, 16 + 4 * q : 16 + 4 * q + 4
                        ],
                        scsb[:tw, 2048 + 512 * q : 2048 + 512 * (q + 1)].rearrange(
                            "t (g s) -> t g s", g=4
                        ),
                    )

                top8 = small.tile([128, 8], F32, tag="top8")
                gidx = small.tile([128, 8], U32, tag="gidx")
                nc.vector.max(top8[:tw], gmax[:tw])
                nc.vector.max_index(gidx[:tw], top8[:tw], gmax[:tw])

                off = small.tile([128, 1], U32, tag="off")
                nc.vector.tensor_tensor(
                    off[:tw], toff32[:tw], gidx[:tw, 0:1], op=ALU.add
                )
                grp = small.tile([128, 128], F32, tag="grp")
                nc.gpsimd.indirect_dma_start(
                    grp[:tw], None, scd[:, :], IndirectOffsetOnAxis(ap=off[:tw], axis=0)
                )
                jidx = small.tile([128, 8], U32, tag="jidx")
                nc.vector.max_index(jidx[:tw], top8[:tw], grp[:tw])
                vidx = small.tile([128, 1], U32, tag="vidx")
                nc.vector.tensor_scalar_mul(vidx[:tw], gidx[:tw, 0:1], 128)
                nc.vector.tensor_tensor(
                    vidx[:tw], vidx[:tw], jidx[:tw, 0:1], op=ALU.add
                )

                htok = small.tile([128, C], F32, tag="htok")
                nc.gpsimd.indirect_dma_start(
                    htok[:tw],
                    None,
                    emb_in,
                    IndirectOffsetOnAxis(ap=vidx[:tw], axis=0),
                )
                # scatter into h_sp [p, (b,c)]: per-b partition-shift copies,
                # spread across engine DMA queues so they don't serialize
                qs = [nc.sync, nc.scalar, nc.gpsimd]
                for j, b in enumerate(range(t0 // P, (t0 + tw) // P)):
                    r0 = b * P - t0
                    qs[j % 3].dma_start(
                        h_sp[0:P, 32 * b : 32 * (b + 1)], htok[r0 : r0 + P]
                    )

            # ---- bicubic up + residual/partial updates ----
            for ch in range(2):
                pu = psum.tile([128, 512], F32, tag="psq", bufs=3)
                nc.tensor.matmul(
                    pu,
                    u_sb[pn][:, 128 * ch : 128 * (ch + 1)],
                    h_sp[:P],
                    start=True,
                    stop=True,
                )
                if si < 3:
                    nc.vector.tensor_tensor(
                        f_rest[ch], f_rest[ch], pu, op=ALU.subtract
                    )
                nc.vector.tensor_tensor(f_partial[ch], f_partial[ch], pu, op=ALU.add)

            # ---- x output ----
            if si < 3:
                pn2 = PNS[si + 1]
                P2 = pn2 * pn2
                px = psum.tile([128, 512], F32, tag="psq", bufs=3)
                for ch in range(2):
                    nc.tensor.matmul(
                        px[:P2],
                        a_sb[pn2][ch][:, :P2],
                        f_partial[ch],
                        start=(ch == 0),
                        stop=(ch == 1),
                    )
                x_sb = small.tile([max(P2, 1), 512], F32, tag="xsb")
                nc.scalar.activation(x_sb[:P2], px[:P2], ACTF.Copy)
                nc.sync.dma_start(x_out[ROW_OFF[si] : ROW_OFF[si] + P2], x_sb[:P2])
            else:
                for ch in range(2):
                    nc.sync.dma_start(
                        x_out[84 + 128 * ch : 84 + 128 * (ch + 1)], f_partial[ch]
                    )

        ctx.close()

    nc.compile()
    return nc


_PROGRAM = None


def _get_program():
    global _PROGRAM
    if _PROGRAM is None:
        _PROGRAM = _build_program()
    return _PROGRAM


def kernel(f_src, emb_weight):
    global LAST_RESULTS
    f_src = np.asarray(f_src, dtype=np.float32)
    emb_weight = np.asarray(emb_weight, dtype=np.float32)

    e64 = emb_weight.astype(np.float64)
    eaug = np.concatenate(
        [emb_weight.T, (-0.5 * (e64 * e64).sum(1)).astype(np.float32)[None, :]], axis=0
    )  # [33, V]

    a_mats = {}
    u_mats = {}
    for pn in PNS:
        P = pn * pn
        a_mats[pn] = np.ascontiguousarray(
            _down_matrix(pn).T.reshape(2, 128, P)
        )  # [2, 128, P]
        u_mats[pn] = np.ascontiguousarray(_up_matrix(pn).T)  # [P, 256]

    in_maps = []
    for core in range(N_CORES):
        fb = f_src[core * B_LOC : (core + 1) * B_LOC]  # [16, 32, 16, 16]
        f_pre = (
            fb.reshape(B_LOC, C, S).transpose(2, 0, 1).reshape(2, 128, 512)
        )  # [s, b, c]
        m = {
            "f_pre": np.ascontiguousarray(f_pre),
            "eaug": np.ascontiguousarray(eaug),
            "embt": np.ascontiguousarray(emb_weight),
        }
        for pn in PNS:
            m[f"a{pn}"] = a_mats[pn]
            m[f"u{pn}"] = u_mats[pn]
        in_maps.append(m)

    nc = _get_program()
    trace = bool(os.environ.get("CVAR_TRACE"))
    try:
        res = run_bass_kernel_spmd(
            nc,
            in_maps,
            core_ids=list(range(N_CORES)),
            trace=trace,
        )
    except ModuleNotFoundError:
        res = run_bass_kernel_spmd(
            nc, in_maps, core_ids=list(range(N_CORES)), trace=False
        )
    LAST_RESULTS = res

    outs = []
    for core in range(N_CORES):
        xo = res.results[core]["xout"]  # [340, 512]
        outs.append(xo.reshape(NTOK_OUT, B_LOC, C).transpose(1, 0, 2))
    return np.ascontiguousarray(np.concatenate(outs, axis=0))


# revision 21
# speedup vs baseline: 1.2644x; 1.0800x over previous
"""CycleVAR VQ-codebook encoder kernel for Trainium2 (8 NeuronCores).

Contract: kernel(**inputs) takes FULL inputs
  f_src      [128, 32, 16, 16] fp32
  emb_weight [4096, 32] fp32
and returns the FULL output x_var [128, 340, 32] fp32.

Observation: the reference's x_var depends only on quantization stages
pn in (1, 2, 4, 8); the pn=16 stage's outputs (stage_maps[4], f_hat,
final f_rest) are never used. So only 85 tokens/image are quantized.

Sharding: data-parallel over batch (16 images per core), codebook and
resize matrices replicated. No cross-core communication.

Per-core pipeline (all layouts per-core, B=16, C=32, H=W=16, S=256):
  f layout "spatial": [s(part, 2x128), (b,c)(free, 512)]
  stage si, pn in (1,2,4,8), P=pn^2, ntok=16P, tokens t = b*P + p:
    z-down   (PE):  z[(b,c), p] = sum_s f_rest[s, bc] * A_pn[p, s]
    zaug     (ACT): per-b copies -> zaug[0:32, b*P:(b+1)*P]; row 32 = 1.0
    scores   (PE):  s[t, v] = sum_k zaug[k, t] * eaug[k, v]  (fp32, K=33,
                    eaug rows 0..31 = emb.T, row 32 = -0.5*|e|^2)
    argmax   (DVE): grouped reduce_max (32 groups of 128) -> max8 ->
                    max_index -> winning group g*; scores roundtrip to
                    DRAM; indirect-DMA gathers row (t*32+g*) -> local
                    max_index -> exact argmax v*
    h        (DMA): indirect gather emb[v*] -> DRAM -> reload as
                    h_sp [p(part), (b,c)]
    up       (PE):  h_up[s, bc] = sum_p U_pn[s, p] * h_sp[p, bc]
    update   (DVE): f_rest -= h_up (si<3); f_partial += h_up
    x_out    (PE):  x_si[p', bc] = sum_s A_pn'[p', s] * f_partial[s, bc]
                    (si<3; si=3 emits f_partial directly)
Output DRAM x_out [340, (b,c)]; host transposes to [b, 340, c].
"""

import os

import numpy as np

import concourse.bacc as bacc
import concourse.bass as bass
import concourse.mybir as mybir
import concourse.tile as tile
from concourse.bass import IndirectOffsetOnAxis
from concourse.bass_utils import run_bass_kernel_spmd

N_CORES = 8
B_FULL = 128
B_LOC = B_FULL // N_CORES  # 16
C = 32
H = 16
S = H * H  # 256
V = 4096
PNS = (1, 2, 4, 8)
ROW_OFF = (0, 4, 20, 84)  # x_var row offsets per stage
NTOK_OUT = 340

F32 = mybir.dt.float32
U32 = mybir.dt.uint32
AX = mybir.AxisListType
ALU = mybir.AluOpType
ACTF = mybir.ActivationFunctionType

LAST_RESULTS = None  # test harness introspection


def _keys_cubic(x, a=-0.5):
    x = np.abs(x)
    return np.where(
        x <= 1,
        (a + 2) * x**3 - (a + 3) * x**2 + 1,
        np.where(x < 2, a * x**3 - 5 * a * x**2 + 8 * a * x - 4 * a, 0.0),
    )


def _resize_matrix_1d(n_in, n_out):
    # matches jax.image.resize(method='cubic') for upsampling
    scale = n_out / n_in
    U = np.zeros((n_out, n_in), np.float64)
    for i in range(n_out):
        x = (i + 0.5) / scale - 0.5
        w = _keys_cubic(x - np.arange(n_in))
        s = w.sum()
        if s != 0:
            w = w / s
        U[i] = w
    return U


def _up_matrix(pn):
    # [S, pn*pn] bicubic upsample matrix (kron of separable 1D)
    if pn == H:
        return np.eye(S, dtype=np.float32)
    U1 = _resize_matrix_1d(pn, H)
    return np.kron(U1, U1).astype(np.float32)


def _down_matrix(pn):
    # [pn*pn, S] exact area mean (r = H//pn, weight 1/r^2, exact pow2)
    r = H // pn
    A = np.zeros((pn * pn, S), np.float32)
    w = np.float32(1.0 / (r * r))
    for pi in range(pn):
        for pj in range(pn):
            for di in range(r):
                for dj in range(r):
                    A[pi * pn + pj, (pi * r + di) * H + (pj * r + dj)] = w
    return A


def _build_program():
    nc = bacc.Bacc(trn_type="TRN2", target_bir_lowering=False, debug=False)

    # DRAM I/O (per core)
    f_in = nc.dram_tensor("f_pre", [2, 128, 512], F32, kind="ExternalInput").ap()
    eaug_in = nc.dram_tensor("eaug", [33, V], F32, kind="ExternalInput").ap()
    emb_in = nc.dram_tensor("embt", [V, C], F32, kind="ExternalInput").ap()
    a_in = {
        pn: nc.dram_tensor(f"a{pn}", [2, 128, pn * pn], F32, kind="ExternalInput").ap()
        for pn in PNS
    }
    u_in = {
        pn: nc.dram_tensor(f"u{pn}", [pn * pn, 256], F32, kind="ExternalInput").ap()
        for pn in PNS
    }
    x_out = nc.dram_tensor("xout", [NTOK_OUT, 512], F32, kind="ExternalOutput").ap()

    with tile.TileContext(nc) as tc:
        from contextlib import ExitStack

        ctx = ExitStack()
        const = ctx.enter_context(tc.tile_pool(name="const", bufs=1))
        work = ctx.enter_context(tc.tile_pool(name="work", bufs=2))
        small = ctx.enter_context(tc.tile_pool(name="small", bufs=2))
        psum = ctx.enter_context(tc.tile_pool(name="psum", bufs=3, space="PSUM"))
        dram = ctx.enter_context(tc.tile_pool(name="dram", bufs=2, space="DRAM"))

        # ---- constants to SBUF ----
        # eaug + zaug replicated at partition base 64 so score matmuls run
        # 2x-packed on PE row-groups {0,1} and {2,3} via tile_position.
        eaug_big = const.tile([97, V], F32)
        nc.sync.dma_start(eaug_big[0:33], eaug_in)
        nc.sync.dma_start(eaug_big[64:97], eaug_in)
        eaug = eaug_big[0:33]
        a_sb = {}
        for pn in PNS:
            P = pn * pn
            a_sb[pn] = [const.tile([128, P], F32, name=f"a{pn}_{ch}") for ch in range(2)]
            for ch in range(2):
                nc.sync.dma_start(a_sb[pn][ch], a_in[pn][ch])
        u_sb = {}
        for pn in PNS:
            P = pn * pn
            u_sb[pn] = const.tile([P, 256], F32, name=f"u{pn}")
            nc.sync.dma_start(u_sb[pn], u_in[pn])

        f_rest = [const.tile([128, 512], F32, name=f"frest{ch}") for ch in range(2)]
        f_partial = [const.tile([128, 512], F32, name=f"fpart{ch}") for ch in range(2)]
        for ch in range(2):
            nc.sync.dma_start(f_rest[ch], f_in[ch])
            nc.vector.memset(f_partial[ch], 0.0)

        zaug_big = const.tile([97, 1024], F32)
        zaug = zaug_big[0:33]
        nc.vector.memset(zaug_big[32:33, :], 1.0)

        toff32 = const.tile([128, 1], U32)  # t*32 per partition
        nc.gpsimd.iota(toff32, pattern=[[1, 1]], base=0, channel_multiplier=32)

        for si, pn in enumerate(PNS):
            P = pn * pn
            ntok = B_LOC * P
            nblk = (ntok + 127) // 128

            # ---- z-down per image: z_b[c, p] = sum_s f_rest[s, (b,c)] A[p, s] ----
            for b in range(B_LOC):
                psz = psum.tile([32, max(P, 1)], F32, tag="psz", bufs=2)
                for ch in range(2):
                    nc.tensor.matmul(
                        psz[:, :P],
                        f_rest[ch][:, 32 * b : 32 * (b + 1)],
                        a_sb[pn][ch][:, :P],
                        start=(ch == 0),
                        stop=(ch == 1),
                    )
                nc.scalar.activation(
                    zaug[0:32, b * P : (b + 1) * P], psz[:, :P], ACTF.Copy
                )
            # replicate tokens (and the ones row) to partition base 64
            half_t = max(8 * P, 1)
            nc.sync.dma_start(zaug_big[64:97, 0:half_t], zaug_big[0:33, 0:half_t])
            nc.scalar.dma_start(
                zaug_big[64:97, half_t:ntok], zaug_big[0:33, half_t:ntok]
            )

            h_sp = work.tile([max(P, 1), 512], F32, tag="hsp", name=f"hsp{si}")

            for blk in range(nblk):
                t0 = blk * 128
                tw = min(128, ntok - t0)

                scsb = work.tile([128, V], F32, tag="scsb", bufs=3)
                gmax = small.tile([128, 32], F32, tag="gmax")
                scd = dram.tile([128 * 32, 128], F32, tag="scd")
                for q in range(4):
                    # 2x-packed: codes [512q, 512q+512) on row-groups 0-1,
                    # codes [2048+512q, ...) on row-groups 2-3 concurrently
                    pq = psum.tile([128, 1024], F32, tag="psq", bufs=3)
                    nc.tensor.matmul(
                        pq[:tw, 0:512],
                        zaug_big[0:33, t0 : t0 + tw],
                        eaug_big[0:33, 512 * q : 512 * (q + 1)],
                        start=True,
                        stop=True,
                        tile_position=(0, 0),
                    )
                    nc.tensor.matmul(
                        pq[:tw, 512:1024],
                        zaug_big[64:97, t0 : t0 + tw],
                        eaug_big[64:97, 2048 + 512 * q : 2048 + 512 * (q + 1)],
                        start=True,
                        stop=True,
                        tile_position=(64, 0),
                    )
                    nc.vector.reduce_max(
                        gmax[:tw, 4 * q : 4 * (q + 1)],
                        pq[:tw, 0:512].rearrange("t (g s) -> t g s", g=4),
                        axis=AX.X,
                    )
                    nc.vector.reduce_max(
                        gmax[:tw, 16 + 4 * q : 16 + 4 * (q + 1)],
                        pq[:tw, 512:1024].rearrange("t (g s) -> t g s", g=4),
                        axis=AX.X,
                    )
                    nc.scalar.activation(
                        scsb[:tw, 512 * q : 512 * (q + 1)], pq[:tw, 0:512], ACTF.Copy
                    )
                    nc.scalar.activation(
                        scsb[:tw, 2048 + 512 * q : 2048 + 512 * (q + 1)],
                        pq[:tw, 512:1024],
                        ACTF.Copy,
                    )
                    nc.sync.dma_start(
                        scd.rearrange("(t g) s -> t g s", g=32)[:tw, 4 * q : 4 * q + 4],
                        scsb[:tw, 512 * q : 512 * (q + 1)].rearrange(
                            "t (g s) -> t g s", g=4
                        ),
                    )
                    nc.sync.dma_start(
                        scd.rearrange("(t g) s -> t g s", g=32)[
                            :tw, 16 + 4 * q : 16 + 4 * q + 4
                        ],
                        scsb[:tw, 2048 + 512 * q : 2048 + 512 * (q + 1)].rearrange(
                            "t (g s) -> t g s", g=4
                        ),
                    )

                top8 = small.tile([128, 8], F32, tag="top8")
                gidx = small.tile([128, 8], U32, tag="gidx")
                nc.vector.max(top8[:tw], gmax[:tw])
                nc.vector.max_index(gidx[:tw], top8[:tw], gmax[:tw])

                off = small.tile([128, 1], U32, tag="off")
                nc.vector.tensor_tensor(
                    off[:tw], toff32[:tw], gidx[:tw, 0:1], op=ALU.add
                )
                grp = small.tile([128, 128], F32, tag="grp")
                nc.gpsimd.indirect_dma_start(
                    grp[:tw], None, scd[:, :], IndirectOffsetOnAxis(ap=off[:tw], axis=0)
                )
                jidx = small.tile([128, 8], U32, tag="jidx")
                nc.vector.max_index(jidx[:tw], top8[:tw], grp[:tw])
                vidx = small.tile([128, 1], U32, tag="vidx")
                nc.vector.tensor_scalar_mul(vidx[:tw], gidx[:tw, 0:1], 128)
                nc.vector.tensor_tensor(
                    vidx[:tw], vidx[:tw], jidx[:tw, 0:1], op=ALU.add
                )

                htok = small.tile([128, C], F32, tag="htok")
                nc.gpsimd.indirect_dma_start(
                    htok[:tw],
                    None,
                    emb_in,
                    IndirectOffsetOnAxis(ap=vidx[:tw], axis=0),
                )
                # scatter into h_sp [p, (b,c)]: per-b partition-shift copies,
                # spread across engine DMA queues so they don't serialize
                qs = [nc.sync, nc.scalar, nc.gpsimd]
                for j, b in enumerate(range(t0 // P, (t0 + tw) // P)):
                    r0 = b * P - t0
                    qs[j % 3].dma_start(
                        h_sp[0:P, 32 * b : 32 * (b + 1)], htok[r0 : r0 + P]
                    )

            # ---- bicubic up + residual/partial updates, split by bc-halves
            # so each half's chain can start as soon as its h rows landed ----
            for hf in range(2):
                cs = slice(256 * hf, 256 * (hf + 1))
                for ch in range(2):
                    pu = psum.tile([128, 256], F32, tag="psq", bufs=3)
                    nc.tensor.matmul(
                        pu,
                        u_sb[pn][:, 128 * ch : 128 * (ch + 1)],
                        h_sp[:P, cs],
                        start=True,
                        stop=True,
                    )
                    if si < 3:
                        nc.vector.tensor_tensor(
                            f_rest[ch][:, cs], f_rest[ch][:, cs], pu, op=ALU.subtract
                        )
                    nc.vector.tensor_tensor(
                        f_partial[ch][:, cs], f_partial[ch][:, cs], pu, op=ALU.add
                    )

            # ---- x output ----
            if si < 3:
                pn2 = PNS[si + 1]
                P2 = pn2 * pn2
                px = psum.tile([128, 512], F32, tag="psq", bufs=3)
                for ch in range(2):
                    nc.tensor.matmul(
                        px[:P2],
                        a_sb[pn2][ch][:, :P2],
                        f_partial[ch],
                        start=(ch == 0),
                        stop=(ch == 1),
                    )
                x_sb = small.tile([max(P2, 1), 512], F32, tag="xsb")
                nc.scalar.activation(x_sb[:P2], px[:P2], ACTF.Copy)
                nc.sync.dma_start(x_out[ROW_OFF[si] : ROW_OFF[si] + P2], x_sb[:P2])
            else:
                for ch in range(2):
                    nc.sync.dma_start(
                        x_out[84 + 128 * ch : 84 + 128 * (ch + 1)], f_partial[ch]
                    )

        ctx.close()

    nc.compile()
    return nc


_PROGRAM = None


def _get_program():
    global _PROGRAM
    if _PROGRAM is None:
        _PROGRAM = _build_program()
    return _PROGRAM


def kernel(f_src, emb_weight):
    global LAST_RESULTS
    f_src = np.asarray(f_src, dtype=np.float32)
    emb_weight = np.asarray(emb_weight, dtype=np.float32)

    e64 = emb_weight.astype(np.float64)
    eaug = np.concatenate(
        [emb_weight.T, (-0.5 * (e64 * e64).sum(1)).astype(np.float32)[None, :]], axis=0
    )  # [33, V]

    a_mats = {}
    u_mats = {}
    for pn in PNS:
        P = pn * pn
        a_mats[pn] = np.ascontiguousarray(
            _down_matrix(pn).T.reshape(2, 128, P)
        )  # [2, 128, P]
        u_mats[pn] = np.ascontiguousarray(_up_matrix(pn).T)  # [P, 256]

    in_maps = []
    for core in range(N_CORES):
        fb = f_src[core * B_LOC : (core + 1) * B_LOC]  # [16, 32, 16, 16]
        f_pre = (
            fb.reshape(B_LOC, C, S).transpose(2, 0, 1).reshape(2, 128, 512)
        )  # [s, b, c]
        m = {
            "f_pre": np.ascontiguousarray(f_pre),
            "eaug": np.ascontiguousarray(eaug),
            "embt": np.ascontiguousarray(emb_weight),
        }
        for pn in PNS:
            m[f"a{pn}"] = a_mats[pn]
            m[f"u{pn}"] = u_mats[pn]
        in_maps.append(m)

    nc = _get_program()
    trace = bool(os.environ.get("CVAR_TRACE"))
    try:
        res = run_bass_kernel_spmd(
            nc,
            in_maps,
            core_ids=list(range(N_CORES)),
            trace=trace,
        )
    except ModuleNotFoundError:
        res = run_bass_kernel_spmd(
            nc, in_maps, core_ids=list(range(N_CORES)), trace=False
        )
    LAST_RESULTS = res

    outs = []
    for core in range(N_CORES):
        xo = res.results[core]["xout"]  # [340, 512]
        outs.append(xo.reshape(NTOK_OUT, B_LOC, C).transpose(1, 0, 2))
    return np.ascontiguousarray(np.concatenate(outs, axis=0))


# revision 22
# speedup vs baseline: 1.2697x; 1.0042x over previous
"""CycleVAR VQ-codebook encoder kernel for Trainium2 (8 NeuronCores).

Contract: kernel(**inputs) takes FULL inputs
  f_src      [128, 32, 16, 16] fp32
  emb_weight [4096, 32] fp32
and returns the FULL output x_var [128, 340, 32] fp32.

Observation: the reference's x_var depends only on quantization stages
pn in (1, 2, 4, 8); the pn=16 stage's outputs (stage_maps[4], f_hat,
final f_rest) are never used. So only 85 tokens/image are quantized.

Sharding: data-parallel over batch (16 images per core), codebook and
resize matrices replicated. No cross-core communication.

Per-core pipeline (all layouts per-core, B=16, C=32, H=W=16, S=256):
  f layout "spatial": [s(part, 2x128), (b,c)(free, 512)]
  stage si, pn in (1,2,4,8), P=pn^2, ntok=16P, tokens t = b*P + p:
    z-down   (PE):  z[(b,c), p] = sum_s f_rest[s, bc] * A_pn[p, s]
    zaug     (ACT): per-b copies -> zaug[0:32, b*P:(b+1)*P]; row 32 = 1.0
    scores   (PE):  s[t, v] = sum_k zaug[k, t] * eaug[k, v]  (fp32, K=33,
                    eaug rows 0..31 = emb.T, row 32 = -0.5*|e|^2)
    argmax   (DVE): grouped reduce_max (32 groups of 128) -> max8 ->
                    max_index -> winning group g*; scores roundtrip to
                    DRAM; indirect-DMA gathers row (t*32+g*) -> local
                    max_index -> exact argmax v*
    h        (DMA): indirect gather emb[v*] -> DRAM -> reload as
                    h_sp [p(part), (b,c)]
    up       (PE):  h_up[s, bc] = sum_p U_pn[s, p] * h_sp[p, bc]
    update   (DVE): f_rest -= h_up (si<3); f_partial += h_up
    x_out    (PE):  x_si[p', bc] = sum_s A_pn'[p', s] * f_partial[s, bc]
                    (si<3; si=3 emits f_partial directly)
Output DRAM x_out [340, (b,c)]; host transposes to [b, 340, c].
"""

import os

import numpy as np

import concourse.bacc as bacc
import concourse.bass as bass
import concourse.mybir as mybir
import concourse.tile as tile
from concourse.bass import IndirectOffsetOnAxis
from concourse.bass_utils import run_bass_kernel_spmd

N_CORES = 8
B_FULL = 128
B_LOC = B_FULL // N_CORES  # 16
C = 32
H = 16
S = H * H  # 256
V = 4096
PNS = (1, 2, 4, 8)
ROW_OFF = (0, 4, 20, 84)  # x_var row offsets per stage
NTOK_OUT = 340

F32 = mybir.dt.float32
U32 = mybir.dt.uint32
AX = mybir.AxisListType
ALU = mybir.AluOpType
ACTF = mybir.ActivationFunctionType

LAST_RESULTS = None  # test harness introspection


def _keys_cubic(x, a=-0.5):
    x = np.abs(x)
    return np.where(
        x <= 1,
        (a + 2) * x**3 - (a + 3) * x**2 + 1,
        np.where(x < 2, a * x**3 - 5 * a * x**2 + 8 * a * x - 4 * a, 0.0),
    )


def _resize_matrix_1d(n_in, n_out):
    # matches jax.image.resize(method='cubic') for upsampling
    scale = n_out / n_in
    U = np.zeros((n_out, n_in), np.float64)
    for i in range(n_out):
        x = (i + 0.5) / scale - 0.5
        w = _keys_cubic(x - np.arange(n_in))
        s = w.sum()
        if s != 0:
            w = w / s
        U[i] = w
    return U


def _up_matrix(pn):
    # [S, pn*pn] bicubic upsample matrix (kron of separable 1D)
    if pn == H:
        return np.eye(S, dtype=np.float32)
    U1 = _resize_matrix_1d(pn, H)
    return np.kron(U1, U1).astype(np.float32)


def _down_matrix(pn):
    # [pn*pn, S] exact area mean (r = H//pn, weight 1/r^2, exact pow2)
    r = H // pn
    A = np.zeros((pn * pn, S), np.float32)
    w = np.float32(1.0 / (r * r))
    for pi in range(pn):
        for pj in range(pn):
            for di in range(r):
                for dj in range(r):
                    A[pi * pn + pj, (pi * r + di) * H + (pj * r + dj)] = w
    return A


def _build_program():
    nc = bacc.Bacc(trn_type="TRN2", target_bir_lowering=False, debug=False)

    # DRAM I/O (per core)
    f_in = nc.dram_tensor("f_pre", [2, 128, 512], F32, kind="ExternalInput").ap()
    eaug_in = nc.dram_tensor("eaug", [33, V], F32, kind="ExternalInput").ap()
    emb_in = nc.dram_tensor("embt", [V, C], F32, kind="ExternalInput").ap()
    a_in = {
        pn: nc.dram_tensor(f"a{pn}", [2, 128, pn * pn], F32, kind="ExternalInput").ap()
        for pn in PNS
    }
    u_in = {
        pn: nc.dram_tensor(f"u{pn}", [pn * pn, 256], F32, kind="ExternalInput").ap()
        for pn in PNS
    }
    x_out = nc.dram_tensor("xout", [NTOK_OUT, 512], F32, kind="ExternalOutput").ap()

    with tile.TileContext(nc) as tc:
        from contextlib import ExitStack

        ctx = ExitStack()
        const = ctx.enter_context(tc.tile_pool(name="const", bufs=1))
        work = ctx.enter_context(tc.tile_pool(name="work", bufs=2))
        small = ctx.enter_context(tc.tile_pool(name="small", bufs=2))
        psum = ctx.enter_context(tc.tile_pool(name="psum", bufs=3, space="PSUM"))
        dram = ctx.enter_context(tc.tile_pool(name="dram", bufs=2, space="DRAM"))

        # ---- constants to SBUF ----
        # eaug + zaug replicated at partition base 64 so score matmuls run
        # 2x-packed on PE row-groups {0,1} and {2,3} via tile_position.
        f_rest = [const.tile([128, 512], F32, name=f"frest{ch}") for ch in range(2)]
        f_partial = [const.tile([128, 512], F32, name=f"fpart{ch}") for ch in range(2)]
        a_sb = {}
        for pn in PNS:
            P = pn * pn
            a_sb[pn] = [const.tile([128, P], F32, name=f"a{pn}_{ch}") for ch in range(2)]
        for ch in range(2):
            nc.sync.dma_start(f_rest[ch], f_in[ch])
        for ch in range(2):
            nc.scalar.dma_start(a_sb[1][ch], a_in[1][ch])
        eaug_big = const.tile([97, V], F32)
        nc.sync.dma_start(eaug_big[0:33], eaug_in)
        nc.scalar.dma_start(eaug_big[64:97], eaug_in)
        eaug = eaug_big[0:33]
        for pn in PNS[1:]:
            for ch in range(2):
                nc.sync.dma_start(a_sb[pn][ch], a_in[pn][ch])
        u_sb = {}
        for pn in PNS:
            P = pn * pn
            u_sb[pn] = const.tile([P, 256], F32, name=f"u{pn}")
            nc.scalar.dma_start(u_sb[pn], u_in[pn])
        for ch in range(2):
            nc.vector.memset(f_partial[ch], 0.0)

        zaug_big = const.tile([97, 1024], F32)
        zaug = zaug_big[0:33]
        nc.vector.memset(zaug_big[32:33, :], 1.0)

        toff32 = const.tile([128, 1], U32)  # t*32 per partition
        nc.gpsimd.iota(toff32, pattern=[[1, 1]], base=0, channel_multiplier=32)

        for si, pn in enumerate(PNS):
            P = pn * pn
            ntok = B_LOC * P
            nblk = (ntok + 127) // 128

            # ---- z-down per image: z_b[c, p] = sum_s f_rest[s, (b,c)] A[p, s] ----
            for b in range(B_LOC):
                psz = psum.tile([32, max(P, 1)], F32, tag="psz", bufs=2)
                for ch in range(2):
                    nc.tensor.matmul(
                        psz[:, :P],
                        f_rest[ch][:, 32 * b : 32 * (b + 1)],
                        a_sb[pn][ch][:, :P],
                        start=(ch == 0),
                        stop=(ch == 1),
                    )
                nc.scalar.activation(
                    zaug[0:32, b * P : (b + 1) * P], psz[:, :P], ACTF.Copy
                )
            # replicate tokens (and the ones row) to partition base 64
            half_t = max(8 * P, 1)
            nc.sync.dma_start(zaug_big[64:97, 0:half_t], zaug_big[0:33, 0:half_t])
            nc.scalar.dma_start(
                zaug_big[64:97, half_t:ntok], zaug_big[0:33, half_t:ntok]
            )

            h_sp = work.tile([max(P, 1), 512], F32, tag="hsp", name=f"hsp{si}")

            for blk in range(nblk):
                t0 = blk * 128
                tw = min(128, ntok - t0)

                scsb = work.tile([128, V], F32, tag="scsb", bufs=3)
                gmax = small.tile([128, 32], F32, tag="gmax")
                scd = dram.tile([128 * 32, 128], F32, tag="scd")
                for q in range(4):
                    # 2x-packed: codes [512q, 512q+512) on row-groups 0-1,
                    # codes [2048+512q, ...) on row-groups 2-3 concurrently
                    pq = psum.tile([128, 1024], F32, tag="psq", bufs=3)
                    nc.tensor.matmul(
                        pq[:tw, 0:512],
                        zaug_big[0:33, t0 : t0 + tw],
                        eaug_big[0:33, 512 * q : 512 * (q + 1)],
                        start=True,
                        stop=True,
                        tile_position=(0, 0),
                    )
                    nc.tensor.matmul(
                        pq[:tw, 512:1024],
                        zaug_big[64:97, t0 : t0 + tw],
                        eaug_big[64:97, 2048 + 512 * q : 2048 + 512 * (q + 1)],
                        start=True,
                        stop=True,
                        tile_position=(64, 0),
                    )
                    nc.vector.reduce_max(
                        gmax[:tw, 4 * q : 4 * (q + 1)],
                        pq[:tw, 0:512].rearrange("t (g s) -> t g s", g=4),
                        axis=AX.X,
                    )
                    nc.vector.reduce_max(
                        gmax[:tw, 16 + 4 * q : 16 + 4 * (q + 1)],
                        pq[:tw, 512:1024].rearrange("t (g s) -> t g s", g=4),
                        axis=AX.X,
                    )
                    nc.scalar.activation(
                        scsb[:tw, 512 * q : 512 * (q + 1)], pq[:tw, 0:512], ACTF.Copy
                    )
                    nc.scalar.activation(
                        scsb[:tw, 2048 + 512 * q : 2048 + 512 * (q + 1)],
                        pq[:tw, 512:1024],
                        ACTF.Copy,
                    )
                    nc.sync.dma_start(
                        scd.rearrange("(t g) s -> t g s", g=32)[:tw, 4 * q : 4 * q + 4],
                        scsb[:tw, 512 * q : 512 * (q + 1)].rearrange(
                            "t (g s) -> t g s", g=4
                        ),
                    )
                    nc.sync.dma_start(
                        scd.rearrange("(t g) s -> t g s", g=32)[
                            :tw, 16 + 4 * q : 16 + 4 * q + 4
                        ],
                        scsb[:tw, 2048 + 512 * q : 2048 + 512 * (q + 1)].rearrange(
                            "t (g s) -> t g s", g=4
                        ),
                    )

                top8 = small.tile([128, 8], F32, tag="top8")
                gidx = small.tile([128, 8], U32, tag="gidx")
                nc.vector.max(top8[:tw], gmax[:tw])
                nc.vector.max_index(gidx[:tw], top8[:tw], gmax[:tw])

                off = small.tile([128, 1], U32, tag="off")
                nc.vector.tensor_tensor(
                    off[:tw], toff32[:tw], gidx[:tw, 0:1], op=ALU.add
                )
                grp = small.tile([128, 128], F32, tag="grp")
                nc.gpsimd.indirect_dma_start(
                    grp[:tw], None, scd[:, :], IndirectOffsetOnAxis(ap=off[:tw], axis=0)
                )
                jidx = small.tile([128, 8], U32, tag="jidx")
                nc.vector.max_index(jidx[:tw], top8[:tw], grp[:tw])
                vidx = small.tile([128, 1], U32, tag="vidx")
                nc.vector.tensor_scalar_mul(vidx[:tw], gidx[:tw, 0:1], 128)
                nc.vector.tensor_tensor(
                    vidx[:tw], vidx[:tw], jidx[:tw, 0:1], op=ALU.add
                )

                htok = small.tile([128, C], F32, tag="htok")
                nc.gpsimd.indirect_dma_start(
                    htok[:tw],
                    None,
                    emb_in,
                    IndirectOffsetOnAxis(ap=vidx[:tw], axis=0),
                )
                # scatter into h_sp [p, (b,c)]: per-b partition-shift copies,
                # spread across engine DMA queues so they don't serialize
                qs = [nc.sync, nc.scalar]
                for j, b in enumerate(range(t0 // P, (t0 + tw) // P)):
                    r0 = b * P - t0
                    qs[j % 2].dma_start(
                        h_sp[0:P, 32 * b : 32 * (b + 1)], htok[r0 : r0 + P]
                    )

            # ---- bicubic up + residual/partial updates, split by bc-halves
            # so each half's chain can start as soon as its h rows landed ----
            for hf in range(2):
                cs = slice(256 * hf, 256 * (hf + 1))
                for ch in range(2):
                    pu = psum.tile([128, 256], F32, tag="psq", bufs=3)
                    nc.tensor.matmul(
                        pu,
                        u_sb[pn][:, 128 * ch : 128 * (ch + 1)],
                        h_sp[:P, cs],
                        start=True,
                        stop=True,
                    )
                    if si < 3:
                        nc.vector.tensor_tensor(
                            f_rest[ch][:, cs], f_rest[ch][:, cs], pu, op=ALU.subtract
                        )
                    nc.vector.tensor_tensor(
                        f_partial[ch][:, cs], f_partial[ch][:, cs], pu, op=ALU.add
                    )

            # ---- x output ----
            if si < 3:
                pn2 = PNS[si + 1]
                P2 = pn2 * pn2
                px = psum.tile([128, 512], F32, tag="psq", bufs=3)
                for ch in range(2):
                    nc.tensor.matmul(
                        px[:P2],
                        a_sb[pn2][ch][:, :P2],
                        f_partial[ch],
                        start=(ch == 0),
                        stop=(ch == 1),
                    )
                x_sb = small.tile([max(P2, 1), 512], F32, tag="xsb")
                nc.scalar.activation(x_sb[:P2], px[:P2], ACTF.Copy)
                nc.sync.dma_start(x_out[ROW_OFF[si] : ROW_OFF[si] + P2], x_sb[:P2])
            else:
                for ch in range(2):
                    for hf in range(2):
                        cs = slice(256 * hf, 256 * (hf + 1))
                        qs2 = [nc.sync, nc.scalar]
                        qs2[hf].dma_start(
                            x_out[84 + 128 * ch : 84 + 128 * (ch + 1), cs],
                            f_partial[ch][:, cs],
                        )

        ctx.close()

    nc.compile()
    return nc


_PROGRAM = None


def _get_program():
    global _PROGRAM
    if _PROGRAM is None:
        _PROGRAM = _build_program()
    return _PROGRAM


def kernel(f_src, emb_weight):
    global LAST_RESULTS
    f_src = np.asarray(f_src, dtype=np.float32)
    emb_weight = np.asarray(emb_weight, dtype=np.float32)

    e64 = emb_weight.astype(np.float64)
    eaug = np.concatenate(
        [emb_weight.T, (-0.5 * (e64 * e64).sum(1)).astype(np.float32)[None, :]], axis=0
    )  # [33, V]

    a_mats = {}
    u_mats = {}
    for pn in PNS:
        P = pn * pn
        a_mats[pn] = np.ascontiguousarray(
            _down_matrix(pn).T.reshape(2, 128, P)
        )  # [2, 128, P]
        u_mats[pn] = np.ascontiguousarray(_up_matrix(pn).T)  # [P, 256]

    in_maps = []
    for core in range(N_CORES):
        fb = f_src[core * B_LOC : (core + 1) * B_LOC]  # [16, 32, 16, 16]
        f_pre = (
            fb.reshape(B_LOC, C, S).transpose(2, 0, 1).reshape(2, 128, 512)
        )  # [s, b, c]
        m = {
            "f_pre": np.ascontiguousarray(f_pre),
            "eaug": np.ascontiguousarray(eaug),
            "embt": np.ascontiguousarray(emb_weight),
        }
        for pn in PNS:
            m[f"a{pn}"] = a_mats[pn]
            m[f"u{pn}"] = u_mats[pn]
        in_maps.append(m)

    nc = _get_program()
    trace = bool(os.environ.get("CVAR_TRACE"))
    try:
        res = run_bass_kernel_spmd(
            nc,
            in_maps,
            core_ids=list(range(N_CORES)),
            trace=trace,
        )
    except ModuleNotFoundError:
        res = run_bass_kernel_spmd(
            nc, in_maps, core_ids=list(range(N_CORES)), trace=False
        )
    LAST_RESULTS = res

    outs = []
    for core in range(N_CORES):
        xo = res.results[core]["xout"]  # [340, 512]
        outs.append(xo.reshape(NTOK_OUT, B_LOC, C).transpose(1, 0, 2))
    return np.ascontiguousarray(np.concatenate(outs, axis=0))
